# revision 2
# baseline (speedup 1.0000x reference)
"""Trainium2 8-core kernel v5 for the GConvGRU-style GNN message-passing net.

Reference computation (N=100000 nodes, E=400000 edges, y = out[:50000]):
    deg  = indeg(dst) + 1;  dinv = rsqrt(deg)
    xs   = D^-1/2 (A + I) D^-1/2 x          # [N, 32] normalized aggregation
    cz   = xs @ Wz + bz ; ch = xs @ Wh + bh # (H == 0 for this problem)
    Z    = sigmoid(cz @ Lz_top + Lz_b); H~ = tanh(ch @ Lh_top + Lh_b)
    Hn   = (1 - Z) * H~
    y    = relu(Hn) @ W_out + b_out         # rows [0, 50000)

v3: no on-device gather.  The host lays every edge payload
(x[src]*dinv[src]*dinv[dst], bf16) into a dense padded slot slab in DRAM:
partition p = band(chunk%4)*32 + feature, free dim = (dst_lane, slot),
slot 0 = self-loop term.  The device streams the slab via HWDGE, does a
contiguous-axis tensor_reduce per 4-chunk group, copies the 4 [32,128]
bands into a zero-masked block-diagonal rhs, and runs 512-col matmuls
with band-replicated folded gate weights.  b_out is added on the host.
"""
import os
import sys

import numpy as np

for _p in ("/root/.axon_site", "/root/.axon_site/_ro/trn_rl_repo",
           "/root/.axon_site/_ro/pypackages", "/opt/trn_rl_repo"):
    if os.path.isdir(_p) and _p not in sys.path:
        sys.path.append(_p)

N = 100000
E = 400000
DIN = 32
FLT = 128
NP_ = 8
NA = 50000
NCORES = 8
NODES_PER_CORE = NA // NCORES          # 6250
P = 128
NCHUNK = (NODES_PER_CORE + P - 1) // P  # 49
NODES_PAD = NCHUNK * P                  # 6272
CH_PER_GRP = 4

_cache = {}


def _split_sync_waits(nc, mybir, limit=1):
    """walrus CoreV3 codegen supports one sync-wait per instruction."""
    cnt = 0
    for fn in nc.m.functions:
        for bb in fn.blocks:
            insts = list(bb.instructions)
            out = []
            changed = False
            for inst in insts:
                si = inst.sync_info
                if si is not None and si.on_wait is not None and len(si.on_wait) > limit:
                    w = list(si.on_wait)
                    upd = list(si.on_update) if si.on_update else []
                    chunks = [w[i:i + limit] for i in range(0, len(w), limit)]
                    for chunk in chunks[:-1]:
                        d = mybir.InstDrain(name=f"I-wsplit{cnt}", ins=[], outs=[])
                        cnt += 1
                        d.engine = inst.engine
                        d.sync_info = mybir.SyncInfo(on_wait=chunk, on_update=[])
                        out.append(d)
                    inst.sync_info = mybir.SyncInfo(on_wait=chunks[-1], on_update=upd)
                    changed = True
                out.append(inst)
            if changed:
                bb.instructions = out


def _build_device_kernel(kg1s, ncg, FTOT):
    """kg1s[g] = slots per node (incl. self) for group g; ncg[g] = chunks in
    group g; FTOT = sum(128*kg1s)."""
    import concourse.bacc as bacc
    import concourse.mybir as mybir
    from concourse.tile import TileContext

    nc = bacc.Bacc("TRN2")
    f32 = mybir.dt.float32
    bf16 = mybir.dt.bfloat16

    NG = len(kg1s)
    slab = nc.declare_dram_parameter("slab", [P, FTOT], bf16, isOutput=False)
    az4 = nc.declare_dram_parameter("az4", [P, FLT], bf16, isOutput=False)
    ah4 = nc.declare_dram_parameter("ah4", [P, FLT], bf16, isOutput=False)
    azn = nc.declare_dram_parameter("azn", [FLT, 1], f32, isOutput=False)
    ahb = nc.declare_dram_parameter("ahb", [FLT, 1], f32, isOutput=False)
    prout = nc.declare_dram_parameter("pr", [P, NODES_PAD], bf16, isOutput=True)

    soff = np.concatenate([[0], np.cumsum([P * k for k in kg1s])]).astype(int)
    goff = np.concatenate([[0], np.cumsum([P * c for c in ncg])]).astype(int)

    with TileContext(nc) as tc:
        with (
            tc.tile_pool(name="const", bufs=1) as cp,
            tc.tile_pool(name="slabp", bufs=1) as sp,
            tc.tile_pool(name="x4p", bufs=4) as xp,
            tc.tile_pool(name="ps", bufs=2, space="PSUM") as pp,
            tc.tile_pool(name="psy", bufs=2, space="PSUM") as pyp,
            tc.tile_pool(name="act", bufs=3) as ap,
        ):
            az_t = cp.tile([P, FLT], bf16)
            nc.scalar.dma_start(out=az_t[:], in_=az4[:, :])
            ah_t = cp.tile([P, FLT], bf16)
            nc.scalar.dma_start(out=ah_t[:], in_=ah4[:, :])
            azn_t = cp.tile([FLT, 1], f32)
            nc.scalar.dma_start(out=azn_t[:], in_=azn[:, :])
            ahb_t = cp.tile([FLT, 1], f32)
            nc.scalar.dma_start(out=ahb_t[:], in_=ahb[:, :])
            rhs = cp.tile([P, NODES_PAD], bf16)
            warm = cp.tile([FLT, 1], f32)
            nc.scalar.activation(out=warm[:], in_=azn_t[:, :1],
                                 func=mybir.ActivationFunctionType.Sigmoid,
                                 bias=ahb_t[:, :1], scale=1.0)
            nc.scalar.activation(out=warm[:], in_=azn_t[:, :1],
                                 func=mybir.ActivationFunctionType.Tanh,
                                 bias=ahb_t[:, :1], scale=1.0)

            # zero the masked rhs per group on gpsimd (off critical path)
            for g in range(NG):
                nc.gpsimd.memset(rhs[:, goff[g]:goff[g + 1]], 0)

            # prefetch all slab group slices on the sync HWDGE ring
            sts = []
            for g in range(NG):
                st = sp.tile([P, P * kg1s[g]], bf16, tag=f"st{g}")
                nc.sync.dma_start(out=st[:], in_=slab[:, soff[g]:soff[g + 1]])
                sts.append(st)

            for g in range(NG):
                k1 = kg1s[g]
                cols = P * ncg[g]
                psz = 32 * ncg[g]
                x4 = xp.tile([P, P], f32, tag="x4")
                nc.vector.tensor_reduce(
                    out=x4[:psz, :],
                    in_=sts[g][:psz, :].rearrange("p (d k) -> p d k", k=k1),
                    axis=mybir.AxisListType.X, op=mybir.AluOpType.add)
                for c in range(ncg[g]):
                    nc.vector.tensor_copy(
                        out=rhs[32 * c:32 * (c + 1),
                                goff[g] + P * c:goff[g] + P * (c + 1)],
                        in_=x4[32 * c:32 * (c + 1), :])

                uz = pp.tile([FLT, P * CH_PER_GRP], f32, tag="uz")
                uh = pp.tile([FLT, P * CH_PER_GRP], f32, tag="uh")
                nc.tensor.matmul(out=uz[:, :cols], lhsT=az_t[:],
                                 rhs=rhs[:, goff[g]:goff[g] + cols],
                                 start=True, stop=True)
                nc.tensor.matmul(out=uh[:, :cols], lhsT=ah_t[:],
                                 rhs=rhs[:, goff[g]:goff[g] + cols],
                                 start=True, stop=True)
                zc = ap.tile([FLT, P * CH_PER_GRP], bf16, tag="zc")
                ht = ap.tile([FLT, P * CH_PER_GRP], bf16, tag="ht")
                nc.scalar.activation(
                    out=zc[:, :cols], in_=uz[:, :cols],
                    func=mybir.ActivationFunctionType.Sigmoid,
                    bias=azn_t[:, :1], scale=-1.0)
                nc.scalar.activation(
                    out=ht[:, :cols], in_=uh[:, :cols],
                    func=mybir.ActivationFunctionType.Tanh,
                    bias=ahb_t[:, :1], scale=1.0)
                pr = ap.tile([FLT, P * CH_PER_GRP], bf16, tag="pr")
                nc.vector.tensor_mul(out=pr[:, :cols], in0=zc[:, :cols],
                                     in1=ht[:, :cols])
                nc.sync.dma_start(out=prout[:, goff[g]:goff[g] + cols],
                                  in_=pr[:, :cols])

    import concourse.mybir as mybir2
    _split_sync_waits(nc, mybir2)
    nc.compile()
    return nc


def _numpy_fallback(x, H, edge_index, Wz, bz, Wr, br, Wh, bh,
                    Lz_w, Lz_b, Lr_w, Lr_b, Lh_w, Lh_b, W_out, b_out):
    """Exact replica of the reference for unexpected inputs (H != 0)."""
    src = np.asarray(edge_index[0], dtype=np.int64)
    dst = np.asarray(edge_index[1], dtype=np.int64)
    deg = np.zeros(N, np.float32)
    np.add.at(deg, dst, 1.0)
    deg += 1.0
    dinv = (1.0 / np.sqrt(deg)).astype(np.float32)

    def gcn(W, b):
        h = x @ W
        norm = (dinv[src] * dinv[dst]).astype(np.float32)
        agg = np.zeros_like(h)
        np.add.at(agg, dst, h[src] * norm[:, None])
        agg = agg + h * (dinv * dinv)[:, None]
        return agg + b

    def sigmoid(v):
        return 1.0 / (1.0 + np.exp(-v))

    cz = gcn(Wz, bz)
    cr = gcn(Wr, br)
    ch = gcn(Wh, bh)
    Z = sigmoid(np.concatenate([cz, H], axis=1) @ Lz_w + Lz_b)
    R = sigmoid(np.concatenate([cr, H], axis=1) @ Lr_w + Lr_b)
    Ht = np.tanh(np.concatenate([ch, H * R], axis=1) @ Lh_w + Lh_b)
    Hn = Z * H + (1.0 - Z) * Ht
    y = np.maximum(Hn, 0.0) @ W_out + b_out
    return y[:NA].astype(np.float32)


def kernel(x, H, edge_index, Wz, bz, Wr, br, Wh, bh,
           Lz_w, Lz_b, Lr_w, Lr_b, Lh_w, Lh_b, W_out, b_out):
    x = np.asarray(x, dtype=np.float32)
    H = np.asarray(H)
    if H.size and np.any(H):
        return _numpy_fallback(x, np.asarray(H, np.float32), edge_index,
                               np.asarray(Wz, np.float32), np.asarray(bz, np.float32),
                               np.asarray(Wr, np.float32), np.asarray(br, np.float32),
                               np.asarray(Wh, np.float32), np.asarray(bh, np.float32),
                               np.asarray(Lz_w, np.float32), np.asarray(Lz_b, np.float32),
                               np.asarray(Lr_w, np.float32), np.asarray(Lr_b, np.float32),
                               np.asarray(Lh_w, np.float32), np.asarray(Lh_b, np.float32),
                               np.asarray(W_out, np.float32), np.asarray(b_out, np.float32))

    src = np.asarray(edge_index[0], dtype=np.int64)
    dst = np.asarray(edge_index[1], dtype=np.int64)

    deg = np.bincount(dst, minlength=N).astype(np.float32) + 1.0
    dinv = (1.0 / np.sqrt(deg)).astype(np.float32)

    # folded gate weights (H = 0 path)
    Wz = np.asarray(Wz, np.float32); Wh = np.asarray(Wh, np.float32)
    Lz_top = np.asarray(Lz_w, np.float32)[:FLT]
    Lh_top = np.asarray(Lh_w, np.float32)[:FLT]
    import ml_dtypes
    bf = ml_dtypes.bfloat16
    Az = (Wz @ Lz_top).astype(np.float32)                       # [32,128]
    Ah = (Wh @ Lh_top).astype(np.float32)
    az = (np.asarray(bz, np.float32) @ Lz_top + np.asarray(Lz_b, np.float32)).astype(np.float32)
    ah = (np.asarray(bh, np.float32) @ Lh_top + np.asarray(Lh_b, np.float32)).astype(np.float32)
    az4 = np.tile(Az, (CH_PER_GRP, 1)).astype(bf)               # [128,128]
    ah4 = np.tile(Ah, (CH_PER_GRP, 1)).astype(bf)
    Wout = np.asarray(W_out, np.float32)                        # [128,8]
    bout = np.asarray(b_out, np.float32)                        # [8]

    # live edges: only dst < NA contribute to the output
    live = dst < NA
    srcL = src[live]
    dstL = dst[live]

    per_core = []
    counts_sorted_all = np.empty((NCORES, NODES_PAD), np.int64)
    for c in range(NCORES):
        lo, hi = c * NODES_PER_CORE, (c + 1) * NODES_PER_CORE
        m = (dstL >= lo) & (dstL < hi)
        s_c = srcL[m]
        d_c = dstL[m] - lo
        cnt = np.bincount(d_c, minlength=NODES_PER_CORE)
        perm = np.argsort(-cnt, kind="stable")
        cs = np.zeros(NODES_PAD, np.int64)
        cs[:NODES_PER_CORE] = cnt[perm]
        counts_sorted_all[c] = cs
        per_core.append((s_c, d_c, cnt, perm))

    # uniform per-chunk slot profile across cores (max in-degree per chunk)
    kprof = np.zeros(NCHUNK, np.int64)
    for ci in range(NCHUNK):
        kprof[ci] = counts_sorted_all[:, ci * P:(ci + 1) * P].max()
    groups = [list(range(g, min(g + CH_PER_GRP, NCHUNK)))
              for g in range(0, NCHUNK, CH_PER_GRP)]
    kg1s = [int(kprof[g].max()) + 1 for g in groups]   # +1 self slot
    ncg = [len(g) for g in groups]
    FTOT = int(sum(P * k for k in kg1s))
    Kmax1 = int(kprof.max()) + 1

    perms = []
    in_maps = []
    for c in range(NCORES):
        s_c, d_c, cnt, perm = per_core[c]
        # position of each node in degree-sorted order
        pos_of = np.empty(NODES_PER_CORE, np.int64)
        pos_of[perm] = np.arange(NODES_PER_CORE)
        order = np.argsort(d_c, kind="stable")
        d_s = d_c[order]
        s_s = s_c[order]
        starts = np.zeros(NODES_PER_CORE + 1, np.int64)
        np.cumsum(cnt, out=starts[1:])
        within = np.arange(len(d_s)) - starts[d_s]

        slab_all = np.zeros((NODES_PAD, Kmax1, DIN), np.float32)
        payload = x[s_s] * (dinv[s_s] * dinv[d_s + c * NODES_PER_CORE])[:, None]
        slab_all[pos_of[d_s], within + 1] = payload
        nodes_perm = perm + c * NODES_PER_CORE
        slab_all[:NODES_PER_CORE, 0] = x[nodes_perm] * (dinv[nodes_perm] ** 2)[:, None]

        # per-group banded layout [band(c)*32+f, dst_lane*k1 + slot]
        slab = np.zeros((P, FTOT), bf)
        off = 0
        for gi, chs in enumerate(groups):
            k1 = kg1s[gi]
            ncg_g = len(chs)
            blk = slab_all[chs[0] * P:(chs[0] + ncg_g) * P, :k1, :]
            # [ncg*128, k1, 32] -> [ncg, 128, k1, 32] -> (c, f, d, k)
            blk = blk.reshape(ncg_g, P, k1, DIN).transpose(0, 3, 1, 2)
            slab[:ncg_g * DIN, off:off + P * k1] = \
                blk.reshape(ncg_g * DIN, P * k1).astype(bf)
            off += P * k1

        perms.append(perm)
        in_maps.append({
            "slab": slab, "az4": az4, "ah4": ah4,
            "azn": (-az).reshape(FLT, 1).astype(np.float32),
            "ahb": ah.reshape(FLT, 1).astype(np.float32),
        })

    if os.environ.get("KERNEL_DEBUG") == "1":
        print(f"[kernel] v5 FTOT={FTOT} kg1s={kg1s} slab={FTOT*P*2/1e6:.2f}MB")
    key = ("v5", tuple(kg1s), tuple(ncg), FTOT)
    if key not in _cache:
        _cache[key] = _build_device_kernel(kg1s, ncg, FTOT)
    nc = _cache[key]

    from concourse.bass_utils import run_bass_kernel_spmd
    trace = os.environ.get("KERNEL_TRACE") == "1"
    kwargs = {}
    if trace:
        kwargs = {"trace": True, "tmpdir": os.environ.get("KERNEL_TRACE_DIR", "/tmp/kernel_trace")}
    res = run_bass_kernel_spmd(nc, in_maps, list(range(NCORES)), **kwargs)
    global last_result
    last_result = res

    y = np.empty((NA, NP_), np.float32)
    for c in range(NCORES):
        prc = res.results[c]["pr"]                    # [128, 6272] bf16
        pr_f = np.maximum(prc[:, :NODES_PER_CORE].T.astype(np.float32), 0.0)
        lo = c * NODES_PER_CORE
        y[lo + perms[c], :] = pr_f @ Wout + bout[None, :]
    return y


# revision 3
# speedup vs baseline: 1.0403x; 1.0403x over previous
"""Trainium2 8-core kernel v7 for the GConvGRU-style GNN message-passing net.

Reference computation (N=100000 nodes, E=400000 edges, y = out[:50000]):
    deg  = indeg(dst) + 1;  dinv = rsqrt(deg)
    xs   = D^-1/2 (A + I) D^-1/2 x          # [N, 32] normalized aggregation
    cz   = xs @ Wz + bz ; ch = xs @ Wh + bh # (H == 0 for this problem)
    Z    = sigmoid(cz @ Lz_top + Lz_b); H~ = tanh(ch @ Lh_top + Lh_b)
    Hn   = (1 - Z) * H~
    y    = relu(Hn) @ W_out + b_out         # rows [0, 50000)

v3: no on-device gather.  The host lays every edge payload
(x[src]*dinv[src]*dinv[dst], bf16) into a dense padded slot slab in DRAM:
partition p = band(chunk%4)*32 + feature, free dim = (dst_lane, slot),
slot 0 = self-loop term.  The device streams the slab via HWDGE, does a
contiguous-axis tensor_reduce per 4-chunk group, copies the 4 [32,128]
bands into a zero-masked block-diagonal rhs, and runs 512-col matmuls
with band-replicated folded gate weights.  b_out is added on the host.
"""
import os
import sys

import numpy as np

for _p in ("/root/.axon_site", "/root/.axon_site/_ro/trn_rl_repo",
           "/root/.axon_site/_ro/pypackages", "/opt/trn_rl_repo"):
    if os.path.isdir(_p) and _p not in sys.path:
        sys.path.append(_p)

N = 100000
E = 400000
DIN = 32
FLT = 128
NP_ = 8
NA = 50000
NCORES = 8
NODES_PER_CORE = NA // NCORES          # 6250
P = 128
NCHUNK = (NODES_PER_CORE + P - 1) // P  # 49
NODES_PAD = NCHUNK * P                  # 6272
CH_PER_GRP = 4

_cache = {}


def _split_sync_waits(nc, mybir, limit=1):
    """walrus CoreV3 codegen supports one sync-wait per instruction."""
    cnt = 0
    for fn in nc.m.functions:
        for bb in fn.blocks:
            insts = list(bb.instructions)
            out = []
            changed = False
            for inst in insts:
                si = inst.sync_info
                if si is not None and si.on_wait is not None and len(si.on_wait) > limit:
                    w = list(si.on_wait)
                    upd = list(si.on_update) if si.on_update else []
                    chunks = [w[i:i + limit] for i in range(0, len(w), limit)]
                    for chunk in chunks[:-1]:
                        d = mybir.InstDrain(name=f"I-wsplit{cnt}", ins=[], outs=[])
                        cnt += 1
                        d.engine = inst.engine
                        d.sync_info = mybir.SyncInfo(on_wait=chunk, on_update=[])
                        out.append(d)
                    inst.sync_info = mybir.SyncInfo(on_wait=chunks[-1], on_update=upd)
                    changed = True
                out.append(inst)
            if changed:
                bb.instructions = out


def _build_device_kernel(kg1s, ncg, FTOT):
    """kg1s[g] = slots per node (incl. self) for group g; ncg[g] = chunks in
    group g; FTOT = sum(128*kg1s)."""
    import concourse.bacc as bacc
    import concourse.mybir as mybir
    from concourse.tile import TileContext

    nc = bacc.Bacc("TRN2")
    f32 = mybir.dt.float32
    bf16 = mybir.dt.bfloat16

    NG = len(kg1s)
    slab = nc.declare_dram_parameter("slab", [P, FTOT], bf16, isOutput=False)
    az4 = nc.declare_dram_parameter("az4", [P, FLT], bf16, isOutput=False)
    ah4 = nc.declare_dram_parameter("ah4", [P, FLT], bf16, isOutput=False)
    azn = nc.declare_dram_parameter("azn", [FLT, 1], f32, isOutput=False)
    ahb = nc.declare_dram_parameter("ahb", [FLT, 1], f32, isOutput=False)
    prout = nc.declare_dram_parameter("pr", [P, NODES_PAD], bf16, isOutput=True)

    soff = np.concatenate([[0], np.cumsum([P * k for k in kg1s])]).astype(int)
    goff = np.concatenate([[0], np.cumsum([P * c for c in ncg])]).astype(int)

    with TileContext(nc) as tc:
        with (
            tc.tile_pool(name="const", bufs=1) as cp,
            tc.tile_pool(name="slabp", bufs=1) as sp,
            tc.tile_pool(name="x4p", bufs=4) as xp,
            tc.tile_pool(name="ps", bufs=2, space="PSUM") as pp,
            tc.tile_pool(name="psy", bufs=2, space="PSUM") as pyp,
            tc.tile_pool(name="act", bufs=3) as ap,
        ):
            az_t = cp.tile([P, FLT], bf16)
            nc.scalar.dma_start(out=az_t[:], in_=az4[:, :])
            ah_t = cp.tile([P, FLT], bf16)
            nc.scalar.dma_start(out=ah_t[:], in_=ah4[:, :])
            azn_t = cp.tile([FLT, 1], f32)
            nc.scalar.dma_start(out=azn_t[:], in_=azn[:, :])
            ahb_t = cp.tile([FLT, 1], f32)
            nc.scalar.dma_start(out=ahb_t[:], in_=ahb[:, :])
            warm = cp.tile([FLT, 1], f32)
            nc.scalar.activation(out=warm[:], in_=azn_t[:, :1],
                                 func=mybir.ActivationFunctionType.Sigmoid,
                                 bias=ahb_t[:, :1], scale=1.0)
            nc.scalar.activation(out=warm[:], in_=azn_t[:, :1],
                                 func=mybir.ActivationFunctionType.Tanh,
                                 bias=ahb_t[:, :1], scale=1.0)



            # prefetch all slab group slices on the sync HWDGE ring
            sts = []
            for g in range(NG):
                st = sp.tile([P, P * kg1s[g]], bf16, tag=f"st{g}")
                nc.sync.dma_start(out=st[:], in_=slab[:, soff[g]:soff[g + 1]])
                sts.append(st)

            for g in range(NG):
                k1 = kg1s[g]
                cols = P * ncg[g]
                psz = 32 * ncg[g]
                x4 = xp.tile([P, P], bf16, tag="x4")
                with nc.allow_low_precision("bf16 xs rounds once post-sum"):
                    nc.vector.tensor_reduce(
                        out=x4[:psz, :],
                        in_=sts[g][:psz, :].rearrange("p (d k) -> p d k", k=k1),
                        axis=mybir.AxisListType.X, op=mybir.AluOpType.add)

                # 4-bank PSUM quad: bank c holds uz_c (cols 0-127) and
                # uh_c (cols 128-255); per-chunk 32x128 row-tiled matmuls
                uq = pp.tile([FLT, 2048], f32, tag="uq")
                for c in range(ncg[g]):
                    nc.tensor.matmul(out=uq[:, 512 * c:512 * c + P],
                                     lhsT=az_t[32 * c:32 * (c + 1), :],
                                     rhs=x4[32 * c:32 * (c + 1), :],
                                     start=True, stop=True,
                                     tile_position=(32 * c, 0),
                                     skip_group_check=True)
                    nc.tensor.matmul(out=uq[:, 512 * c + P:512 * c + 2 * P],
                                     lhsT=ah_t[32 * c:32 * (c + 1), :],
                                     rhs=x4[32 * c:32 * (c + 1), :],
                                     start=True, stop=True,
                                     tile_position=(32 * c, 0),
                                     skip_group_check=True)
                zc = ap.tile([FLT, P * CH_PER_GRP], bf16, tag="zc")
                ht = ap.tile([FLT, P * CH_PER_GRP], bf16, tag="ht")
                uqv = uq[:, :512 * ncg[g]].rearrange(
                    "p (c two d) -> p c two d", two=4, d=P)
                uzv = uqv[:, :, 0:1, :]
                uhv = uqv[:, :, 1:2, :]
                nc.scalar.activation(
                    out=zc[:, :cols], in_=uzv,
                    func=mybir.ActivationFunctionType.Sigmoid,
                    bias=azn_t[:, :1], scale=-1.0)
                nc.scalar.activation(
                    out=ht[:, :cols], in_=uhv,
                    func=mybir.ActivationFunctionType.Tanh,
                    bias=ahb_t[:, :1], scale=1.0)
                pr = ap.tile([FLT, P * CH_PER_GRP], bf16, tag="pr")
                nc.vector.tensor_mul(out=pr[:, :cols], in0=zc[:, :cols],
                                     in1=ht[:, :cols])
                nc.sync.dma_start(out=prout[:, goff[g]:goff[g] + cols],
                                  in_=pr[:, :cols])

    import concourse.mybir as mybir2
    _split_sync_waits(nc, mybir2)
    nc.compile()
    return nc


def _numpy_fallback(x, H, edge_index, Wz, bz, Wr, br, Wh, bh,
                    Lz_w, Lz_b, Lr_w, Lr_b, Lh_w, Lh_b, W_out, b_out):
    """Exact replica of the reference for unexpected inputs (H != 0)."""
    src = np.asarray(edge_index[0], dtype=np.int64)
    dst = np.asarray(edge_index[1], dtype=np.int64)
    deg = np.zeros(N, np.float32)
    np.add.at(deg, dst, 1.0)
    deg += 1.0
    dinv = (1.0 / np.sqrt(deg)).astype(np.float32)

    def gcn(W, b):
        h = x @ W
        norm = (dinv[src] * dinv[dst]).astype(np.float32)
        agg = np.zeros_like(h)
        np.add.at(agg, dst, h[src] * norm[:, None])
        agg = agg + h * (dinv * dinv)[:, None]
        return agg + b

    def sigmoid(v):
        return 1.0 / (1.0 + np.exp(-v))

    cz = gcn(Wz, bz)
    cr = gcn(Wr, br)
    ch = gcn(Wh, bh)
    Z = sigmoid(np.concatenate([cz, H], axis=1) @ Lz_w + Lz_b)
    R = sigmoid(np.concatenate([cr, H], axis=1) @ Lr_w + Lr_b)
    Ht = np.tanh(np.concatenate([ch, H * R], axis=1) @ Lh_w + Lh_b)
    Hn = Z * H + (1.0 - Z) * Ht
    y = np.maximum(Hn, 0.0) @ W_out + b_out
    return y[:NA].astype(np.float32)


def kernel(x, H, edge_index, Wz, bz, Wr, br, Wh, bh,
           Lz_w, Lz_b, Lr_w, Lr_b, Lh_w, Lh_b, W_out, b_out):
    x = np.asarray(x, dtype=np.float32)
    H = np.asarray(H)
    if H.size and np.any(H):
        return _numpy_fallback(x, np.asarray(H, np.float32), edge_index,
                               np.asarray(Wz, np.float32), np.asarray(bz, np.float32),
                               np.asarray(Wr, np.float32), np.asarray(br, np.float32),
                               np.asarray(Wh, np.float32), np.asarray(bh, np.float32),
                               np.asarray(Lz_w, np.float32), np.asarray(Lz_b, np.float32),
                               np.asarray(Lr_w, np.float32), np.asarray(Lr_b, np.float32),
                               np.asarray(Lh_w, np.float32), np.asarray(Lh_b, np.float32),
                               np.asarray(W_out, np.float32), np.asarray(b_out, np.float32))

    src = np.asarray(edge_index[0], dtype=np.int64)
    dst = np.asarray(edge_index[1], dtype=np.int64)

    deg = np.bincount(dst, minlength=N).astype(np.float32) + 1.0
    dinv = (1.0 / np.sqrt(deg)).astype(np.float32)

    # folded gate weights (H = 0 path)
    Wz = np.asarray(Wz, np.float32); Wh = np.asarray(Wh, np.float32)
    Lz_top = np.asarray(Lz_w, np.float32)[:FLT]
    Lh_top = np.asarray(Lh_w, np.float32)[:FLT]
    import ml_dtypes
    bf = ml_dtypes.bfloat16
    Az = (Wz @ Lz_top).astype(np.float32)                       # [32,128]
    Ah = (Wh @ Lh_top).astype(np.float32)
    az = (np.asarray(bz, np.float32) @ Lz_top + np.asarray(Lz_b, np.float32)).astype(np.float32)
    ah = (np.asarray(bh, np.float32) @ Lh_top + np.asarray(Lh_b, np.float32)).astype(np.float32)
    az4 = np.tile(Az, (CH_PER_GRP, 1)).astype(bf)               # [128,128]
    ah4 = np.tile(Ah, (CH_PER_GRP, 1)).astype(bf)
    Wout = np.asarray(W_out, np.float32)                        # [128,8]
    bout = np.asarray(b_out, np.float32)                        # [8]

    # live edges: only dst < NA contribute to the output
    live = dst < NA
    srcL = src[live]
    dstL = dst[live]

    per_core = []
    counts_sorted_all = np.empty((NCORES, NODES_PAD), np.int64)
    for c in range(NCORES):
        lo, hi = c * NODES_PER_CORE, (c + 1) * NODES_PER_CORE
        m = (dstL >= lo) & (dstL < hi)
        s_c = srcL[m]
        d_c = dstL[m] - lo
        cnt = np.bincount(d_c, minlength=NODES_PER_CORE)
        perm = np.argsort(-cnt, kind="stable")
        cs = np.zeros(NODES_PAD, np.int64)
        cs[:NODES_PER_CORE] = cnt[perm]
        counts_sorted_all[c] = cs
        per_core.append((s_c, d_c, cnt, perm))

    # uniform per-chunk slot profile across cores (max in-degree per chunk)
    kprof = np.zeros(NCHUNK, np.int64)
    for ci in range(NCHUNK):
        kprof[ci] = counts_sorted_all[:, ci * P:(ci + 1) * P].max()
    groups = [list(range(g, min(g + CH_PER_GRP, NCHUNK)))
              for g in range(0, NCHUNK, CH_PER_GRP)]
    kg1s = [int(kprof[g].max()) + 1 for g in groups]   # +1 self slot
    ncg = [len(g) for g in groups]
    FTOT = int(sum(P * k for k in kg1s))
    Kmax1 = int(kprof.max()) + 1

    perms = []
    in_maps = []
    for c in range(NCORES):
        s_c, d_c, cnt, perm = per_core[c]
        # position of each node in degree-sorted order
        pos_of = np.empty(NODES_PER_CORE, np.int64)
        pos_of[perm] = np.arange(NODES_PER_CORE)
        order = np.argsort(d_c, kind="stable")
        d_s = d_c[order]
        s_s = s_c[order]
        starts = np.zeros(NODES_PER_CORE + 1, np.int64)
        np.cumsum(cnt, out=starts[1:])
        within = np.arange(len(d_s)) - starts[d_s]

        slab_all = np.zeros((NODES_PAD, Kmax1, DIN), np.float32)
        payload = x[s_s] * (dinv[s_s] * dinv[d_s + c * NODES_PER_CORE])[:, None]
        slab_all[pos_of[d_s], within + 1] = payload
        nodes_perm = perm + c * NODES_PER_CORE
        slab_all[:NODES_PER_CORE, 0] = x[nodes_perm] * (dinv[nodes_perm] ** 2)[:, None]

        # per-group banded layout [band(c)*32+f, dst_lane*k1 + slot]
        slab = np.zeros((P, FTOT), bf)
        off = 0
        for gi, chs in enumerate(groups):
            k1 = kg1s[gi]
            ncg_g = len(chs)
            blk = slab_all[chs[0] * P:(chs[0] + ncg_g) * P, :k1, :]
            # [ncg*128, k1, 32] -> [ncg, 128, k1, 32] -> (c, f, d, k)
            blk = blk.reshape(ncg_g, P, k1, DIN).transpose(0, 3, 1, 2)
            slab[:ncg_g * DIN, off:off + P * k1] = \
                blk.reshape(ncg_g * DIN, P * k1).astype(bf)
            off += P * k1

        perms.append(perm)
        in_maps.append({
            "slab": slab, "az4": az4, "ah4": ah4,
            "azn": (-az).reshape(FLT, 1).astype(np.float32),
            "ahb": ah.reshape(FLT, 1).astype(np.float32),
        })

    if os.environ.get("KERNEL_DEBUG") == "1":
        print(f"[kernel] v7 FTOT={FTOT} kg1s={kg1s} slab={FTOT*P*2/1e6:.2f}MB")
    key = ("v7", tuple(kg1s), tuple(ncg), FTOT)
    if key not in _cache:
        _cache[key] = _build_device_kernel(kg1s, ncg, FTOT)
    nc = _cache[key]

    from concourse.bass_utils import run_bass_kernel_spmd
    trace = os.environ.get("KERNEL_TRACE") == "1"
    kwargs = {}
    if trace:
        kwargs = {"trace": True, "tmpdir": os.environ.get("KERNEL_TRACE_DIR", "/tmp/kernel_trace")}
    res = run_bass_kernel_spmd(nc, in_maps, list(range(NCORES)), **kwargs)
    global last_result
    last_result = res

    y = np.empty((NA, NP_), np.float32)
    for c in range(NCORES):
        prc = res.results[c]["pr"]                    # [128, 6272] bf16
        pr_f = np.maximum(prc[:, :NODES_PER_CORE].T.astype(np.float32), 0.0)
        lo = c * NODES_PER_CORE
        y[lo + perms[c], :] = pr_f @ Wout + bout[None, :]
    return y


# revision 4
# speedup vs baseline: 1.0830x; 1.0411x over previous
"""Trainium2 8-core kernel v8 for the GConvGRU-style GNN message-passing net.

Reference computation (N=100000 nodes, E=400000 edges, y = out[:50000]):
    deg  = indeg(dst) + 1;  dinv = rsqrt(deg)
    xs   = D^-1/2 (A + I) D^-1/2 x          # [N, 32] normalized aggregation
    cz   = xs @ Wz + bz ; ch = xs @ Wh + bh # (H == 0 for this problem)
    Z    = sigmoid(cz @ Lz_top + Lz_b); H~ = tanh(ch @ Lh_top + Lh_b)
    Hn   = (1 - Z) * H~
    y    = relu(Hn) @ W_out + b_out         # rows [0, 50000)

v3: no on-device gather.  The host lays every edge payload
(x[src]*dinv[src]*dinv[dst], bf16) into a dense padded slot slab in DRAM:
partition p = band(chunk%4)*32 + feature, free dim = (dst_lane, slot),
slot 0 = self-loop term.  The device streams the slab via HWDGE, does a
contiguous-axis tensor_reduce per 4-chunk group, copies the 4 [32,128]
bands into a zero-masked block-diagonal rhs, and runs 512-col matmuls
with band-replicated folded gate weights.  b_out is added on the host.
"""
import os
import sys

import numpy as np

for _p in ("/root/.axon_site", "/root/.axon_site/_ro/trn_rl_repo",
           "/root/.axon_site/_ro/pypackages", "/opt/trn_rl_repo"):
    if os.path.isdir(_p) and _p not in sys.path:
        sys.path.append(_p)

N = 100000
E = 400000
DIN = 32
FLT = 128
NP_ = 8
NA = 50000
NCORES = 8
NODES_PER_CORE = NA // NCORES          # 6250
P = 128
CHUNK = 256                             # dst nodes per chunk (matmul cols)
NCHUNK = (NODES_PER_CORE + CHUNK - 1) // CHUNK  # 25
NODES_PAD = NCHUNK * CHUNK              # 6400
CH_PER_GRP = 4

_cache = {}


def _split_sync_waits(nc, mybir, limit=1):
    """walrus CoreV3 codegen supports one sync-wait per instruction."""
    cnt = 0
    for fn in nc.m.functions:
        for bb in fn.blocks:
            insts = list(bb.instructions)
            out = []
            changed = False
            for inst in insts:
                si = inst.sync_info
                if si is not None and si.on_wait is not None and len(si.on_wait) > limit:
                    w = list(si.on_wait)
                    upd = list(si.on_update) if si.on_update else []
                    chunks = [w[i:i + limit] for i in range(0, len(w), limit)]
                    for chunk in chunks[:-1]:
                        d = mybir.InstDrain(name=f"I-wsplit{cnt}", ins=[], outs=[])
                        cnt += 1
                        d.engine = inst.engine
                        d.sync_info = mybir.SyncInfo(on_wait=chunk, on_update=[])
                        out.append(d)
                    inst.sync_info = mybir.SyncInfo(on_wait=chunks[-1], on_update=upd)
                    changed = True
                out.append(inst)
            if changed:
                bb.instructions = out


def _build_device_kernel(kg1s, ncg, FTOT):
    """kg1s[g] = slots per node (incl. self) for group g; ncg[g] = chunks in
    group g; FTOT = sum(128*kg1s)."""
    import concourse.bacc as bacc
    import concourse.mybir as mybir
    from concourse.tile import TileContext

    nc = bacc.Bacc("TRN2")
    f32 = mybir.dt.float32
    bf16 = mybir.dt.bfloat16

    NG = len(kg1s)
    slab = nc.declare_dram_parameter("slab", [P, FTOT], bf16, isOutput=False)
    az4 = nc.declare_dram_parameter("az4", [P, FLT], bf16, isOutput=False)
    ah4 = nc.declare_dram_parameter("ah4", [P, FLT], bf16, isOutput=False)
    azn = nc.declare_dram_parameter("azn", [FLT, 1], f32, isOutput=False)
    ahb = nc.declare_dram_parameter("ahb", [FLT, 1], f32, isOutput=False)
    prout = nc.declare_dram_parameter("pr", [P, NODES_PAD], bf16, isOutput=True)

    soff = np.concatenate([[0], np.cumsum([CHUNK * k for k in kg1s])]).astype(int)
    goff = np.concatenate([[0], np.cumsum([CHUNK * c for c in ncg])]).astype(int)

    with TileContext(nc) as tc:
        with (
            tc.tile_pool(name="const", bufs=1) as cp,
            tc.tile_pool(name="slabp", bufs=1) as sp,
            tc.tile_pool(name="x4p", bufs=4) as xp,
            tc.tile_pool(name="ps", bufs=2, space="PSUM") as pp,
            tc.tile_pool(name="psy", bufs=2, space="PSUM") as pyp,
            tc.tile_pool(name="act", bufs=3) as ap,
        ):
            az_t = cp.tile([P, FLT], bf16)
            nc.scalar.dma_start(out=az_t[:], in_=az4[:, :])
            ah_t = cp.tile([P, FLT], bf16)
            nc.scalar.dma_start(out=ah_t[:], in_=ah4[:, :])
            azn_t = cp.tile([FLT, 1], f32)
            nc.scalar.dma_start(out=azn_t[:], in_=azn[:, :])
            ahb_t = cp.tile([FLT, 1], f32)
            nc.scalar.dma_start(out=ahb_t[:], in_=ahb[:, :])
            warm = cp.tile([FLT, 1], f32)
            nc.scalar.activation(out=warm[:], in_=azn_t[:, :1],
                                 func=mybir.ActivationFunctionType.Sigmoid,
                                 bias=ahb_t[:, :1], scale=1.0)
            nc.scalar.activation(out=warm[:], in_=azn_t[:, :1],
                                 func=mybir.ActivationFunctionType.Tanh,
                                 bias=ahb_t[:, :1], scale=1.0)



            # prefetch all slab group slices on the sync HWDGE ring
            sts = []
            for g in range(NG):
                st = sp.tile([P, CHUNK * kg1s[g]], bf16, tag=f"st{g}")
                nc.sync.dma_start(out=st[:], in_=slab[:, soff[g]:soff[g + 1]])
                sts.append(st)

            for g in range(NG):
                k1 = kg1s[g]
                cols = CHUNK * ncg[g]
                psz = 32 * ncg[g]
                x4 = xp.tile([P, CHUNK], bf16, tag="x4")
                with nc.allow_low_precision("bf16 xs rounds once post-sum"):
                    nc.vector.tensor_reduce(
                        out=x4[:psz, :],
                        in_=sts[g][:psz, :].rearrange("p (d k) -> p d k", k=k1),
                        axis=mybir.AxisListType.X, op=mybir.AluOpType.add)

                # 4-bank PSUM quad: bank c holds uz_c (cols 0-127) and
                # uh_c (cols 128-255); per-chunk 32x128 row-tiled matmuls
                uq = pp.tile([FLT, 2048], f32, tag="uq")
                for c in range(ncg[g]):
                    nc.tensor.matmul(out=uq[:, 512 * c:512 * c + CHUNK],
                                     lhsT=az_t[32 * c:32 * (c + 1), :],
                                     rhs=x4[32 * c:32 * (c + 1), :],
                                     start=True, stop=True,
                                     tile_position=(32 * c, 0),
                                     skip_group_check=True)
                    nc.tensor.matmul(out=uq[:, 512 * c + CHUNK:512 * (c + 1)],
                                     lhsT=ah_t[32 * c:32 * (c + 1), :],
                                     rhs=x4[32 * c:32 * (c + 1), :],
                                     start=True, stop=True,
                                     tile_position=(32 * c, 0),
                                     skip_group_check=True)
                zc = ap.tile([FLT, CHUNK * CH_PER_GRP], bf16, tag="zc")
                ht = ap.tile([FLT, CHUNK * CH_PER_GRP], bf16, tag="ht")
                uqv = uq[:, :512 * ncg[g]].rearrange(
                    "p (c two d) -> p c two d", two=2, d=CHUNK)
                uzv = uqv[:, :, 0:1, :]
                uhv = uqv[:, :, 1:2, :]
                nc.scalar.activation(
                    out=zc[:, :cols], in_=uzv,
                    func=mybir.ActivationFunctionType.Sigmoid,
                    bias=azn_t[:, :1], scale=-1.0)
                nc.scalar.activation(
                    out=ht[:, :cols], in_=uhv,
                    func=mybir.ActivationFunctionType.Tanh,
                    bias=ahb_t[:, :1], scale=1.0)
                pr = ap.tile([FLT, CHUNK * CH_PER_GRP], bf16, tag="pr")
                nc.vector.tensor_mul(out=pr[:, :cols], in0=zc[:, :cols],
                                     in1=ht[:, :cols])
                nc.sync.dma_start(out=prout[:, goff[g]:goff[g] + cols],
                                  in_=pr[:, :cols])

    import concourse.mybir as mybir2
    _split_sync_waits(nc, mybir2)
    nc.compile()
    return nc


def _numpy_fallback(x, H, edge_index, Wz, bz, Wr, br, Wh, bh,
                    Lz_w, Lz_b, Lr_w, Lr_b, Lh_w, Lh_b, W_out, b_out):
    """Exact replica of the reference for unexpected inputs (H != 0)."""
    src = np.asarray(edge_index[0], dtype=np.int64)
    dst = np.asarray(edge_index[1], dtype=np.int64)
    deg = np.zeros(N, np.float32)
    np.add.at(deg, dst, 1.0)
    deg += 1.0
    dinv = (1.0 / np.sqrt(deg)).astype(np.float32)

    def gcn(W, b):
        h = x @ W
        norm = (dinv[src] * dinv[dst]).astype(np.float32)
        agg = np.zeros_like(h)
        np.add.at(agg, dst, h[src] * norm[:, None])
        agg = agg + h * (dinv * dinv)[:, None]
        return agg + b

    def sigmoid(v):
        return 1.0 / (1.0 + np.exp(-v))

    cz = gcn(Wz, bz)
    cr = gcn(Wr, br)
    ch = gcn(Wh, bh)
    Z = sigmoid(np.concatenate([cz, H], axis=1) @ Lz_w + Lz_b)
    R = sigmoid(np.concatenate([cr, H], axis=1) @ Lr_w + Lr_b)
    Ht = np.tanh(np.concatenate([ch, H * R], axis=1) @ Lh_w + Lh_b)
    Hn = Z * H + (1.0 - Z) * Ht
    y = np.maximum(Hn, 0.0) @ W_out + b_out
    return y[:NA].astype(np.float32)


def kernel(x, H, edge_index, Wz, bz, Wr, br, Wh, bh,
           Lz_w, Lz_b, Lr_w, Lr_b, Lh_w, Lh_b, W_out, b_out):
    x = np.asarray(x, dtype=np.float32)
    H = np.asarray(H)
    if H.size and np.any(H):
        return _numpy_fallback(x, np.asarray(H, np.float32), edge_index,
                               np.asarray(Wz, np.float32), np.asarray(bz, np.float32),
                               np.asarray(Wr, np.float32), np.asarray(br, np.float32),
                               np.asarray(Wh, np.float32), np.asarray(bh, np.float32),
                               np.asarray(Lz_w, np.float32), np.asarray(Lz_b, np.float32),
                               np.asarray(Lr_w, np.float32), np.asarray(Lr_b, np.float32),
                               np.asarray(Lh_w, np.float32), np.asarray(Lh_b, np.float32),
                               np.asarray(W_out, np.float32), np.asarray(b_out, np.float32))

    src = np.asarray(edge_index[0], dtype=np.int64)
    dst = np.asarray(edge_index[1], dtype=np.int64)

    deg = np.bincount(dst, minlength=N).astype(np.float32) + 1.0
    dinv = (1.0 / np.sqrt(deg)).astype(np.float32)

    # folded gate weights (H = 0 path)
    Wz = np.asarray(Wz, np.float32); Wh = np.asarray(Wh, np.float32)
    Lz_top = np.asarray(Lz_w, np.float32)[:FLT]
    Lh_top = np.asarray(Lh_w, np.float32)[:FLT]
    import ml_dtypes
    bf = ml_dtypes.bfloat16
    Az = (Wz @ Lz_top).astype(np.float32)                       # [32,128]
    Ah = (Wh @ Lh_top).astype(np.float32)
    az = (np.asarray(bz, np.float32) @ Lz_top + np.asarray(Lz_b, np.float32)).astype(np.float32)
    ah = (np.asarray(bh, np.float32) @ Lh_top + np.asarray(Lh_b, np.float32)).astype(np.float32)
    az4 = np.tile(Az, (CH_PER_GRP, 1)).astype(bf)               # [128,128]
    ah4 = np.tile(Ah, (CH_PER_GRP, 1)).astype(bf)
    Wout = np.asarray(W_out, np.float32)                        # [128,8]
    bout = np.asarray(b_out, np.float32)                        # [8]

    # live edges: only dst < NA contribute to the output
    live = dst < NA
    srcL = src[live]
    dstL = dst[live]

    per_core = []
    counts_sorted_all = np.empty((NCORES, NODES_PAD), np.int64)
    for c in range(NCORES):
        lo, hi = c * NODES_PER_CORE, (c + 1) * NODES_PER_CORE
        m = (dstL >= lo) & (dstL < hi)
        s_c = srcL[m]
        d_c = dstL[m] - lo
        cnt = np.bincount(d_c, minlength=NODES_PER_CORE)
        perm = np.argsort(-cnt, kind="stable")
        cs = np.zeros(NODES_PAD, np.int64)
        cs[:NODES_PER_CORE] = cnt[perm]
        counts_sorted_all[c] = cs
        per_core.append((s_c, d_c, cnt, perm))

    # uniform per-chunk slot profile across cores (max in-degree per chunk)
    kprof = np.zeros(NCHUNK, np.int64)
    for ci in range(NCHUNK):
        kprof[ci] = counts_sorted_all[:, ci * CHUNK:(ci + 1) * CHUNK].max()
    groups = [list(range(g, min(g + CH_PER_GRP, NCHUNK)))
              for g in range(0, NCHUNK, CH_PER_GRP)]
    kg1s = [int(kprof[g].max()) + 1 for g in groups]   # +1 self slot
    ncg = [len(g) for g in groups]
    FTOT = int(sum(CHUNK * k for k in kg1s))
    Kmax1 = int(kprof.max()) + 1

    perms = []
    in_maps = []
    for c in range(NCORES):
        s_c, d_c, cnt, perm = per_core[c]
        # position of each node in degree-sorted order
        pos_of = np.empty(NODES_PER_CORE, np.int64)
        pos_of[perm] = np.arange(NODES_PER_CORE)
        order = np.argsort(d_c, kind="stable")
        d_s = d_c[order]
        s_s = s_c[order]
        starts = np.zeros(NODES_PER_CORE + 1, np.int64)
        np.cumsum(cnt, out=starts[1:])
        within = np.arange(len(d_s)) - starts[d_s]

        slab_all = np.zeros((NODES_PAD, Kmax1, DIN), np.float32)
        payload = x[s_s] * (dinv[s_s] * dinv[d_s + c * NODES_PER_CORE])[:, None]
        slab_all[pos_of[d_s], within + 1] = payload
        nodes_perm = perm + c * NODES_PER_CORE
        slab_all[:NODES_PER_CORE, 0] = x[nodes_perm] * (dinv[nodes_perm] ** 2)[:, None]

        # per-group banded layout [band(c)*32+f, dst_lane*k1 + slot]
        slab = np.zeros((P, FTOT), bf)
        off = 0
        for gi, chs in enumerate(groups):
            k1 = kg1s[gi]
            ncg_g = len(chs)
            blk = slab_all[chs[0] * CHUNK:(chs[0] + ncg_g) * CHUNK, :k1, :]
            # [ncg*CHUNK, k1, 32] -> [ncg, CHUNK, k1, 32] -> (c, f, d, k)
            blk = blk.reshape(ncg_g, CHUNK, k1, DIN).transpose(0, 3, 1, 2)
            slab[:ncg_g * DIN, off:off + CHUNK * k1] = \
                blk.reshape(ncg_g * DIN, CHUNK * k1).astype(bf)
            off += CHUNK * k1

        perms.append(perm)
        in_maps.append({
            "slab": slab, "az4": az4, "ah4": ah4,
            "azn": (-az).reshape(FLT, 1).astype(np.float32),
            "ahb": ah.reshape(FLT, 1).astype(np.float32),
        })

    if os.environ.get("KERNEL_DEBUG") == "1":
        print(f"[kernel] v8 FTOT={FTOT} kg1s={kg1s} slab={FTOT*P*2/1e6:.2f}MB")
    key = ("v8", tuple(kg1s), tuple(ncg), FTOT)
    if key not in _cache:
        _cache[key] = _build_device_kernel(kg1s, ncg, FTOT)
    nc = _cache[key]

    from concourse.bass_utils import run_bass_kernel_spmd
    trace = os.environ.get("KERNEL_TRACE") == "1"
    kwargs = {}
    if trace:
        kwargs = {"trace": True, "tmpdir": os.environ.get("KERNEL_TRACE_DIR", "/tmp/kernel_trace")}
    res = run_bass_kernel_spmd(nc, in_maps, list(range(NCORES)), **kwargs)
    global last_result
    last_result = res

    y = np.empty((NA, NP_), np.float32)
    for c in range(NCORES):
        prc = res.results[c]["pr"]                    # [128, 6272] bf16
        pr_f = np.maximum(prc[:, :NODES_PER_CORE].T.astype(np.float32), 0.0)
        lo = c * NODES_PER_CORE
        y[lo + perms[c], :] = pr_f @ Wout + bout[None, :]
    return y


# revision 5
# speedup vs baseline: 1.2170x; 1.1237x over previous
"""Trainium2 8-core kernel v9 for the GConvGRU-style GNN message-passing net.

Reference computation (N=100000 nodes, E=400000 edges, y = out[:50000]):
    deg  = indeg(dst) + 1;  dinv = rsqrt(deg)
    xs   = D^-1/2 (A + I) D^-1/2 x          # [N, 32] normalized aggregation
    cz   = xs @ Wz + bz ; ch = xs @ Wh + bh # (H == 0 for this problem)
    Z    = sigmoid(cz @ Lz_top + Lz_b); H~ = tanh(ch @ Lh_top + Lh_b)
    Hn   = (1 - Z) * H~
    y    = relu(Hn) @ W_out + b_out         # rows [0, 50000)

v3: no on-device gather.  The host lays every edge payload
(x[src]*dinv[src]*dinv[dst], bf16) into a dense padded slot slab in DRAM:
partition p = band(chunk%4)*32 + feature, free dim = (dst_lane, slot),
slot 0 = self-loop term.  The device streams the slab via HWDGE, does a
contiguous-axis tensor_reduce per 4-chunk group, copies the 4 [32,128]
bands into a zero-masked block-diagonal rhs, and runs 512-col matmuls
with band-replicated folded gate weights.  b_out is added on the host.
"""
import os
import sys

import numpy as np

for _p in ("/root/.axon_site", "/root/.axon_site/_ro/trn_rl_repo",
           "/root/.axon_site/_ro/pypackages", "/opt/trn_rl_repo"):
    if os.path.isdir(_p) and _p not in sys.path:
        sys.path.append(_p)

N = 100000
E = 400000
DIN = 32
FLT = 128
NP_ = 8
NA = 50000
NCORES = 8
NODES_PER_CORE = NA // NCORES          # 6250
P = 128
CHUNK = 256                             # dst nodes per chunk (matmul cols)
NCHUNK = (NODES_PER_CORE + CHUNK - 1) // CHUNK  # 25
NODES_PAD = NCHUNK * CHUNK              # 6400
CH_PER_GRP = 4

_cache = {}


def _split_sync_waits(nc, mybir, limit=1):
    """walrus CoreV3 codegen supports one sync-wait per instruction."""
    cnt = 0
    for fn in nc.m.functions:
        for bb in fn.blocks:
            insts = list(bb.instructions)
            out = []
            changed = False
            for inst in insts:
                si = inst.sync_info
                if si is not None and si.on_wait is not None and len(si.on_wait) > limit:
                    w = list(si.on_wait)
                    upd = list(si.on_update) if si.on_update else []
                    chunks = [w[i:i + limit] for i in range(0, len(w), limit)]
                    for chunk in chunks[:-1]:
                        d = mybir.InstDrain(name=f"I-wsplit{cnt}", ins=[], outs=[])
                        cnt += 1
                        d.engine = inst.engine
                        d.sync_info = mybir.SyncInfo(on_wait=chunk, on_update=[])
                        out.append(d)
                    inst.sync_info = mybir.SyncInfo(on_wait=chunks[-1], on_update=upd)
                    changed = True
                out.append(inst)
            if changed:
                bb.instructions = out


def _build_device_kernel(kg1s, ncg, FTOT):
    """kg1s[g] = slots per node (incl. self) for group g; ncg[g] = chunks in
    group g; FTOT = sum(128*kg1s)."""
    import concourse.bacc as bacc
    import concourse.mybir as mybir
    from concourse.tile import TileContext

    nc = bacc.Bacc("TRN2")
    f32 = mybir.dt.float32
    bf16 = mybir.dt.bfloat16

    NG = len(kg1s)
    slab = nc.declare_dram_parameter("slab", [P, FTOT], bf16, isOutput=False)
    az4 = nc.declare_dram_parameter("az4", [P, FLT], bf16, isOutput=False)
    ah4 = nc.declare_dram_parameter("ah4", [P, FLT], bf16, isOutput=False)
    azn = nc.declare_dram_parameter("azn", [FLT, 1], f32, isOutput=False)
    ahb = nc.declare_dram_parameter("ahb", [FLT, 1], f32, isOutput=False)
    prout = nc.declare_dram_parameter("pr", [P, NODES_PAD], bf16, isOutput=True)

    soff = np.concatenate([[0], np.cumsum([CHUNK * k for k in kg1s])]).astype(int)
    goff = np.concatenate([[0], np.cumsum([CHUNK * c for c in ncg])]).astype(int)

    order = sorted(range(NG), key=lambda g: kg1s[g])

    with TileContext(nc) as tc:
        with (
            tc.tile_pool(name="const", bufs=1) as cp,
            tc.tile_pool(name="slabp", bufs=1) as sp,
            tc.tile_pool(name="x4p", bufs=4) as xp,
            tc.tile_pool(name="ps", bufs=2, space="PSUM") as pp,
            tc.tile_pool(name="psy", bufs=2, space="PSUM") as pyp,
            tc.tile_pool(name="act", bufs=3) as ap,
        ):
            az_t = cp.tile([P, FLT], bf16)
            nc.scalar.dma_start(out=az_t[:], in_=az4[:, :])
            ah_t = cp.tile([P, FLT], bf16)
            nc.scalar.dma_start(out=ah_t[:], in_=ah4[:, :])
            azn_t = cp.tile([FLT, 1], f32)
            nc.scalar.dma_start(out=azn_t[:], in_=azn[:, :])
            ahb_t = cp.tile([FLT, 1], f32)
            nc.scalar.dma_start(out=ahb_t[:], in_=ahb[:, :])
            warm = cp.tile([FLT, 1], f32)
            nc.scalar.activation(out=warm[:], in_=azn_t[:, :1],
                                 func=mybir.ActivationFunctionType.Sigmoid,
                                 bias=ahb_t[:, :1], scale=1.0)
            nc.scalar.activation(out=warm[:], in_=azn_t[:, :1],
                                 func=mybir.ActivationFunctionType.Tanh,
                                 bias=ahb_t[:, :1], scale=1.0)



            # prefetch all slab group slices on the sync HWDGE ring,
            # smallest group first so the pipeline fills fast
            sts = {}
            for g in order:
                st = sp.tile([P, CHUNK * kg1s[g]], bf16, tag=f"st{g}",
                             name=f"st{g}")
                nc.sync.dma_start(out=st[:], in_=slab[:, soff[g]:soff[g + 1]])
                sts[g] = st

            for g in order:
                k1 = kg1s[g]
                cols = CHUNK * ncg[g]
                psz = 32 * ncg[g]
                x4 = xp.tile([P, CHUNK], bf16, tag="x4")
                with nc.allow_low_precision("bf16 xs rounds once post-sum"):
                    nc.vector.tensor_reduce(
                        out=x4[:psz, :],
                        in_=sts[g][:psz, :].rearrange("p (d k) -> p d k", k=k1),
                        axis=mybir.AxisListType.X, op=mybir.AluOpType.add)

                # 4-bank PSUM quad: bank c holds uz_c (cols 0-127) and
                # uh_c (cols 128-255); per-chunk 32x128 row-tiled matmuls
                uq = pp.tile([FLT, 2048], f32, tag="uq")
                for c in range(ncg[g]):
                    nc.tensor.matmul(out=uq[:, 512 * c:512 * c + CHUNK],
                                     lhsT=az_t[32 * c:32 * (c + 1), :],
                                     rhs=x4[32 * c:32 * (c + 1), :],
                                     start=True, stop=True,
                                     tile_position=(32 * c, 0),
                                     skip_group_check=True)
                    nc.tensor.matmul(out=uq[:, 512 * c + CHUNK:512 * (c + 1)],
                                     lhsT=ah_t[32 * c:32 * (c + 1), :],
                                     rhs=x4[32 * c:32 * (c + 1), :],
                                     start=True, stop=True,
                                     tile_position=(32 * c, 0),
                                     skip_group_check=True)
                zc = ap.tile([FLT, CHUNK * CH_PER_GRP], bf16, tag="zc")
                ht = ap.tile([FLT, CHUNK * CH_PER_GRP], bf16, tag="ht")
                uqv = uq[:, :512 * ncg[g]].rearrange(
                    "p (c two d) -> p c two d", two=2, d=CHUNK)
                uzv = uqv[:, :, 0:1, :]
                uhv = uqv[:, :, 1:2, :]
                nc.scalar.activation(
                    out=zc[:, :cols], in_=uzv,
                    func=mybir.ActivationFunctionType.Sigmoid,
                    bias=azn_t[:, :1], scale=-1.0)
                nc.scalar.activation(
                    out=ht[:, :cols], in_=uhv,
                    func=mybir.ActivationFunctionType.Tanh,
                    bias=ahb_t[:, :1], scale=1.0)
                pr = ap.tile([FLT, CHUNK * CH_PER_GRP], bf16, tag="pr")
                nc.vector.tensor_mul(out=pr[:, :cols], in0=zc[:, :cols],
                                     in1=ht[:, :cols])
                nc.sync.dma_start(out=prout[:, goff[g]:goff[g] + cols],
                                  in_=pr[:, :cols])

    import concourse.mybir as mybir2
    _split_sync_waits(nc, mybir2)
    nc.compile()
    return nc


def _numpy_fallback(x, H, edge_index, Wz, bz, Wr, br, Wh, bh,
                    Lz_w, Lz_b, Lr_w, Lr_b, Lh_w, Lh_b, W_out, b_out):
    """Exact replica of the reference for unexpected inputs (H != 0)."""
    src = np.asarray(edge_index[0], dtype=np.int64)
    dst = np.asarray(edge_index[1], dtype=np.int64)
    deg = np.zeros(N, np.float32)
    np.add.at(deg, dst, 1.0)
    deg += 1.0
    dinv = (1.0 / np.sqrt(deg)).astype(np.float32)

    def gcn(W, b):
        h = x @ W
        norm = (dinv[src] * dinv[dst]).astype(np.float32)
        agg = np.zeros_like(h)
        np.add.at(agg, dst, h[src] * norm[:, None])
        agg = agg + h * (dinv * dinv)[:, None]
        return agg + b

    def sigmoid(v):
        return 1.0 / (1.0 + np.exp(-v))

    cz = gcn(Wz, bz)
    cr = gcn(Wr, br)
    ch = gcn(Wh, bh)
    Z = sigmoid(np.concatenate([cz, H], axis=1) @ Lz_w + Lz_b)
    R = sigmoid(np.concatenate([cr, H], axis=1) @ Lr_w + Lr_b)
    Ht = np.tanh(np.concatenate([ch, H * R], axis=1) @ Lh_w + Lh_b)
    Hn = Z * H + (1.0 - Z) * Ht
    y = np.maximum(Hn, 0.0) @ W_out + b_out
    return y[:NA].astype(np.float32)


def kernel(x, H, edge_index, Wz, bz, Wr, br, Wh, bh,
           Lz_w, Lz_b, Lr_w, Lr_b, Lh_w, Lh_b, W_out, b_out):
    x = np.asarray(x, dtype=np.float32)
    H = np.asarray(H)
    if H.size and np.any(H):
        return _numpy_fallback(x, np.asarray(H, np.float32), edge_index,
                               np.asarray(Wz, np.float32), np.asarray(bz, np.float32),
                               np.asarray(Wr, np.float32), np.asarray(br, np.float32),
                               np.asarray(Wh, np.float32), np.asarray(bh, np.float32),
                               np.asarray(Lz_w, np.float32), np.asarray(Lz_b, np.float32),
                               np.asarray(Lr_w, np.float32), np.asarray(Lr_b, np.float32),
                               np.asarray(Lh_w, np.float32), np.asarray(Lh_b, np.float32),
                               np.asarray(W_out, np.float32), np.asarray(b_out, np.float32))

    src = np.asarray(edge_index[0], dtype=np.int64)
    dst = np.asarray(edge_index[1], dtype=np.int64)

    deg = np.bincount(dst, minlength=N).astype(np.float32) + 1.0
    dinv = (1.0 / np.sqrt(deg)).astype(np.float32)

    # folded gate weights (H = 0 path)
    Wz = np.asarray(Wz, np.float32); Wh = np.asarray(Wh, np.float32)
    Lz_top = np.asarray(Lz_w, np.float32)[:FLT]
    Lh_top = np.asarray(Lh_w, np.float32)[:FLT]
    import ml_dtypes
    bf = ml_dtypes.bfloat16
    Az = (Wz @ Lz_top).astype(np.float32)                       # [32,128]
    Ah = (Wh @ Lh_top).astype(np.float32)
    az = (np.asarray(bz, np.float32) @ Lz_top + np.asarray(Lz_b, np.float32)).astype(np.float32)
    ah = (np.asarray(bh, np.float32) @ Lh_top + np.asarray(Lh_b, np.float32)).astype(np.float32)
    az4 = np.tile(Az, (CH_PER_GRP, 1)).astype(bf)               # [128,128]
    ah4 = np.tile(Ah, (CH_PER_GRP, 1)).astype(bf)
    Wout = np.asarray(W_out, np.float32)                        # [128,8]
    bout = np.asarray(b_out, np.float32)                        # [8]

    # live edges: only dst < NA contribute to the output
    live = dst < NA
    srcL = src[live]
    dstL = dst[live]

    per_core = []
    counts_sorted_all = np.empty((NCORES, NODES_PAD), np.int64)
    for c in range(NCORES):
        lo, hi = c * NODES_PER_CORE, (c + 1) * NODES_PER_CORE
        m = (dstL >= lo) & (dstL < hi)
        s_c = srcL[m]
        d_c = dstL[m] - lo
        cnt = np.bincount(d_c, minlength=NODES_PER_CORE)
        perm = np.argsort(-cnt, kind="stable")
        cs = np.zeros(NODES_PAD, np.int64)
        cs[:NODES_PER_CORE] = cnt[perm]
        counts_sorted_all[c] = cs
        per_core.append((s_c, d_c, cnt, perm))

    # uniform per-chunk slot profile across cores (max in-degree per chunk)
    kprof = np.zeros(NCHUNK, np.int64)
    for ci in range(NCHUNK):
        kprof[ci] = counts_sorted_all[:, ci * CHUNK:(ci + 1) * CHUNK].max()
    groups = [list(range(g, min(g + CH_PER_GRP, NCHUNK)))
              for g in range(0, NCHUNK, CH_PER_GRP)]
    kg1s = [int(kprof[g].max()) + 1 for g in groups]   # +1 self slot
    ncg = [len(g) for g in groups]
    FTOT = int(sum(CHUNK * k for k in kg1s))
    Kmax1 = int(kprof.max()) + 1

    perms = []
    in_maps = []
    for c in range(NCORES):
        s_c, d_c, cnt, perm = per_core[c]
        # position of each node in degree-sorted order
        pos_of = np.empty(NODES_PER_CORE, np.int64)
        pos_of[perm] = np.arange(NODES_PER_CORE)
        order = np.argsort(d_c, kind="stable")
        d_s = d_c[order]
        s_s = s_c[order]
        starts = np.zeros(NODES_PER_CORE + 1, np.int64)
        np.cumsum(cnt, out=starts[1:])
        within = np.arange(len(d_s)) - starts[d_s]

        slab_all = np.zeros((NODES_PAD, Kmax1, DIN), np.float32)
        payload = x[s_s] * (dinv[s_s] * dinv[d_s + c * NODES_PER_CORE])[:, None]
        slab_all[pos_of[d_s], within + 1] = payload
        nodes_perm = perm + c * NODES_PER_CORE
        slab_all[:NODES_PER_CORE, 0] = x[nodes_perm] * (dinv[nodes_perm] ** 2)[:, None]

        # per-group banded layout [band(c)*32+f, dst_lane*k1 + slot]
        slab = np.zeros((P, FTOT), bf)
        off = 0
        for gi, chs in enumerate(groups):
            k1 = kg1s[gi]
            ncg_g = len(chs)
            blk = slab_all[chs[0] * CHUNK:(chs[0] + ncg_g) * CHUNK, :k1, :]
            # [ncg*CHUNK, k1, 32] -> [ncg, CHUNK, k1, 32] -> (c, f, d, k)
            blk = blk.reshape(ncg_g, CHUNK, k1, DIN).transpose(0, 3, 1, 2)
            slab[:ncg_g * DIN, off:off + CHUNK * k1] = \
                blk.reshape(ncg_g * DIN, CHUNK * k1).astype(bf)
            off += CHUNK * k1

        perms.append(perm)
        in_maps.append({
            "slab": slab, "az4": az4, "ah4": ah4,
            "azn": (-az).reshape(FLT, 1).astype(np.float32),
            "ahb": ah.reshape(FLT, 1).astype(np.float32),
        })

    if os.environ.get("KERNEL_DEBUG") == "1":
        print(f"[kernel] v9 FTOT={FTOT} kg1s={kg1s} slab={FTOT*P*2/1e6:.2f}MB")
    key = ("v9", tuple(kg1s), tuple(ncg), FTOT)
    if key not in _cache:
        _cache[key] = _build_device_kernel(kg1s, ncg, FTOT)
    nc = _cache[key]

    from concourse.bass_utils import run_bass_kernel_spmd
    trace = os.environ.get("KERNEL_TRACE") == "1"
    kwargs = {}
    if trace:
        kwargs = {"trace": True, "tmpdir": os.environ.get("KERNEL_TRACE_DIR", "/tmp/kernel_trace")}
    res = run_bass_kernel_spmd(nc, in_maps, list(range(NCORES)), **kwargs)
    global last_result
    last_result = res

    y = np.empty((NA, NP_), np.float32)
    for c in range(NCORES):
        prc = res.results[c]["pr"]                    # [128, 6272] bf16
        pr_f = np.maximum(prc[:, :NODES_PER_CORE].T.astype(np.float32), 0.0)
        lo = c * NODES_PER_CORE
        y[lo + perms[c], :] = pr_f @ Wout + bout[None, :]
    return y
